# revision 8
# baseline (speedup 1.0000x reference)
"""KGTN (GGNN-propagated classifier) Trainium2 kernel, 8-core SPMD.

Sharding: the 2048 GGNN nodes (classes) are split 256/core. Each GGNN step
needs the full node state for aggregation, so the bf16 node state is
all-gathered between steps (2 AllGathers total — the last step's update only
needs local rows). The final classifier x @ weight is column-sharded: core j
computes output[:, j*256:(j+1)*256].T from its own propagated weight shard,
so no gather of the weight is needed. Matmuls run in bf16 with fp32 PSUM
accumulation; the node state itself is kept in fp32.
"""

import numpy as np
import ml_dtypes

import concourse.bass as bass
import concourse.mybir as mybir
import concourse.tile as tile
from concourse import bacc, bass_utils

P = 128
NCORES = 8
N = 2048          # num_classes (GGNN nodes)
H = 1024          # feature dim
B = 8192          # batch
SH = N // NCORES  # 256 node rows per core
KN = N // P       # 16
KH = H // P       # 8
SM = SH // P      # 2
TSTEPS = 3
NB = 512          # classifier batch chunk

BF = mybir.dt.bfloat16
F32 = mybir.dt.float32
bf16 = ml_dtypes.bfloat16

_CACHE = {}


def _build(for_sim=False):
    nc = bacc.Bacc(num_devices=1 if for_sim else NCORES)

    # per-core inputs (pre-sharded / pre-transposed on host)
    inb_d = nc.dram_tensor("inb", [N, 2 * SH], BF, kind="ExternalInput")      # [inT_j | incols_j]
    nodes0_d = nc.dram_tensor("nodes0", [N, H], BF, kind="ExternalInput")     # init nodes, full
    st0_d = nc.dram_tensor("st0", [SH, H], F32, kind="ExternalInput")         # local init rows, f32
    n0T_d = nc.dram_tensor("n0T", [H, SH], BF, kind="ExternalInput")          # local init rows, T
    w3w_d = nc.dram_tensor("w3w", [N, H], BF, kind="ExternalInput")
    w4w_d = nc.dram_tensor("w4w", [N, H], BF, kind="ExternalInput")
    w5w_d = nc.dram_tensor("w5w", [N, H], BF, kind="ExternalInput")
    w3u_d = nc.dram_tensor("w3u", [H, H], BF, kind="ExternalInput")
    w4u_d = nc.dram_tensor("w4u", [H, H], BF, kind="ExternalInput")
    w5u_d = nc.dram_tensor("w5u", [H, H], BF, kind="ExternalInput")
    fcw_d = nc.dram_tensor("fcw", [N, H], BF, kind="ExternalInput")           # fc_out_w
    fcb_d = nc.dram_tensor("fcb", [P, KH], F32, kind="ExternalInput")         # fc_out_b as [p, m]
    xT_d = nc.dram_tensor("xT", [H, B], BF, kind="ExternalInput")             # x.T, full
    outT_d = nc.dram_tensor("outT", [SH, B], F32, kind="ExternalOutput")      # output cols shard, T
    wout_d = nc.dram_tensor("wout", [H, SH], F32, kind="ExternalOutput")      # weight cols shard

    RG = [list(range(NCORES))]

    with tile.TileContext(nc) as tc:
        with (
            tc.tile_pool(name="const", bufs=1) as const,
            tc.tile_pool(name="dram", bufs=1, space="DRAM") as dram,
            tc.tile_pool(name="wstream", bufs=12) as wstream,
            tc.tile_pool(name="apool", bufs=2) as apool,
            tc.tile_pool(name="work", bufs=2) as work,
        ):
            # ---- resident constants
            inb_s = const.tile([P, KN, 2 * SH], BF)
            nc.sync.dma_start(inb_s[:], inb_d.rearrange("(k p) s -> p k s", p=P))
            n0T_s = const.tile([P, KH, SH], BF)
            nc.sync.dma_start(n0T_s[:], n0T_d.rearrange("(k p) s -> p k s", p=P))
            fcb_s = const.tile([P, KH], F32)
            nc.sync.dma_start(fcb_s[:], fcb_d[:])
            identF = const.tile([P, P], F32)
            from concourse.masks import make_identity
            make_identity(nc, identF)

            state = const.tile([P, SM, H], F32, name="state0")
            nc.sync.dma_start(state[:], st0_d.rearrange("(m p) h -> p m h", p=P))
            stateT = n0T_s  # nodes_t transposed (bf16); step 0 uses init directly

            ag_in = [dram.tile([SH, H], BF, name=f"agin{t}") for t in range(2)]
            ag_out = [
                dram.tile([N, H], BF, addr_space="Shared", name=f"agout{t}")
                for t in range(2)
            ]

            for t in range(TSTEPS):
                # ======== aggregation: avT[2h, SH] = [(in_j@nodes).T ; (out_j@nodes).T]
                avT = apool.tile([P, KN, SH], BF, tag="avT", bufs=1, name=f"avT{t}")
                nall = apool.tile([P, KN, H], BF, tag="nall", bufs=1, name=f"nall{t}")
                src = nodes0_d if t == 0 else ag_out[t - 1]
                for k in range(KN):
                    nc.sync.dma_start(nall[:, k, :], src[k * P:(k + 1) * P, :])
                with tc.tile_pool(name=f"psA{t}", bufs=2, space="PSUM") as psA:
                    for m in range(KH):
                        y = psA.tile([P, 2 * SH], F32, tag="y", name=f"y{t}_{m}")
                        for k in range(KN):
                            nc.tensor.matmul(
                                y[:],
                                lhsT=nall[:, k, m * P:(m + 1) * P],
                                rhs=inb_s[:, k, :],
                                start=(k == 0),
                                stop=(k == KN - 1),
                            )
                        nc.vector.tensor_copy(avT[:, m, :], y[:, :SH])
                        nc.vector.tensor_copy(avT[:, KH + m, :], y[:, SH:])

                # ======== gates (natural layout, rows shard)
                staten = work.tile([P, SM, H], F32, tag="staten", name=f"staten{t}")
                stateTn = work.tile([P, KH, SH], BF, tag="stateTn", name=f"stateTn{t}")
                nbf = work.tile([P, SM, H], BF, tag="nbf", name=f"nbf{t}")
                with tc.tile_pool(name=f"psG{t}", bufs=1, space="PSUM") as psG:
                    for mr in range(SM):
                        zvp = psG.tile([P, H], F32, tag="zvp", name=f"zvp{t}_{mr}")
                        rvp = psG.tile([P, H], F32, tag="rvp", name=f"rvp{t}_{mr}")
                        hvp = psG.tile([P, H], F32, tag="hvp", name=f"hvp{t}_{mr}")
                        mrs = slice(mr * P, (mr + 1) * P)
                        # u-part for zv, rv first: contracts local nodes_t^T,
                        # independent of the AllGather -> overlaps with it
                        for k in range(KH):
                            w3ut = wstream.tile([P, H], BF, tag="wt", name=f"w3ut{t}_{mr}_{k}")
                            nc.sync.dma_start(w3ut[:], w3u_d[k * P:(k + 1) * P, :])
                            w4ut = wstream.tile([P, H], BF, tag="wt", name=f"w4ut{t}_{mr}_{k}")
                            nc.sync.dma_start(w4ut[:], w4u_d[k * P:(k + 1) * P, :])
                            lhs = stateT[:, k, mrs]
                            for nh in range(2):
                                ns = slice(nh * 512, (nh + 1) * 512)
                                nc.tensor.matmul(zvp[:, ns], lhsT=lhs, rhs=w3ut[:, ns],
                                                 start=(k == 0), stop=False)
                                nc.tensor.matmul(rvp[:, ns], lhsT=lhs, rhs=w4ut[:, ns],
                                                 start=(k == 0), stop=False)
                        # av-part: contract over 2h, closes zv/rv groups
                        for k in range(KN):
                            w3t = wstream.tile([P, H], BF, tag="wt", name=f"w3t{t}_{mr}_{k}")
                            nc.sync.dma_start(w3t[:], w3w_d[k * P:(k + 1) * P, :])
                            w4t = wstream.tile([P, H], BF, tag="wt", name=f"w4t{t}_{mr}_{k}")
                            nc.sync.dma_start(w4t[:], w4w_d[k * P:(k + 1) * P, :])
                            w5t = wstream.tile([P, H], BF, tag="wt", name=f"w5t{t}_{mr}_{k}")
                            nc.sync.dma_start(w5t[:], w5w_d[k * P:(k + 1) * P, :])
                            lhs = avT[:, k, mrs]
                            last = (k == KN - 1)
                            for nh in range(2):
                                ns = slice(nh * 512, (nh + 1) * 512)
                                nc.tensor.matmul(zvp[:, ns], lhsT=lhs, rhs=w3t[:, ns],
                                                 start=False, stop=last)
                                nc.tensor.matmul(rvp[:, ns], lhsT=lhs, rhs=w4t[:, ns],
                                                 start=False, stop=last)
                                nc.tensor.matmul(hvp[:, ns], lhsT=lhs, rhs=w5t[:, ns],
                                                 start=(k == 0), stop=False)
                        zv = work.tile([P, H], F32, tag="zv", bufs=1, name=f"zv{t}_{mr}")
                        rv = work.tile([P, H], F32, tag="rv", bufs=1, name=f"rv{t}_{mr}")
                        nc.scalar.activation(zv[:], zvp[:], mybir.ActivationFunctionType.Sigmoid)
                        nc.scalar.activation(rv[:], rvp[:], mybir.ActivationFunctionType.Sigmoid)
                        # rvn = rv * nodes_t  (fp32), transposed to bf16 for U5
                        rvn = work.tile([P, H], F32, tag="rvn", bufs=1, name=f"rvn{t}_{mr}")
                        nc.vector.tensor_tensor(rvn[:], rv[:], state[:, mr, :],
                                                mybir.AluOpType.mult)
                        rvnT = work.tile([P, KH, P], BF, tag="rvnT", bufs=1, name=f"rvnT{t}_{mr}")
                        for k in range(KH):
                            tp = psG.tile([P, P], F32, tag="tp", bufs=2, name=f"tp{t}_{mr}_{k}")
                            nc.tensor.transpose(tp[:], rvn[:, k * P:(k + 1) * P], identF[:])
                            nc.vector.tensor_copy(rvnT[:, k, :], tp[:])
                        for k in range(KH):
                            w5ut = wstream.tile([P, H], BF, tag="wt", name=f"w5ut{t}_{mr}_{k}")
                            nc.sync.dma_start(w5ut[:], w5u_d[k * P:(k + 1) * P, :])
                            for nh in range(2):
                                ns = slice(nh * 512, (nh + 1) * 512)
                                nc.tensor.matmul(hvp[:, ns], lhsT=rvnT[:, k, :],
                                                 rhs=w5ut[:, ns],
                                                 start=False, stop=(k == KH - 1))
                        hv = work.tile([P, H], F32, tag="hv", bufs=1, name=f"hv{t}_{mr}")
                        nc.scalar.activation(hv[:], hvp[:], mybir.ActivationFunctionType.Tanh)
                        # nodes_{t+1} = nodes + zv*(hv - nodes)
                        nc.vector.tensor_tensor(hv[:], hv[:], state[:, mr, :],
                                                mybir.AluOpType.subtract)
                        nc.vector.tensor_tensor(hv[:], zv[:], hv[:], mybir.AluOpType.mult)
                        nc.vector.tensor_tensor(staten[:, mr, :], state[:, mr, :], hv[:],
                                                mybir.AluOpType.add)
                        if t < TSTEPS - 1:
                            nc.vector.tensor_copy(nbf[:, mr, :], staten[:, mr, :])
                        # transposed new state (bf16) for next-step u-parts / final
                        for k in range(KH):
                            tp2 = psG.tile([P, P], F32, tag="tp", bufs=2, name=f"tp2{t}_{mr}_{k}")
                            nc.tensor.transpose(tp2[:], staten[:, mr, k * P:(k + 1) * P],
                                                identF[:])
                            nc.vector.tensor_copy(stateTn[:, k, mrs], tp2[:])

                state = staten
                stateT = stateTn
                if t < TSTEPS - 1:
                    nc.sync.dma_start(ag_in[t].rearrange("(m p) h -> p m h", p=P), nbf[:])
                    if for_sim:
                        nc.sync.dma_start(ag_out[t][0:SH, :], ag_in[t][:])
                    else:
                        nc.gpsimd.collective_compute(
                            "AllGather", mybir.AluOpType.bypass, replica_groups=RG,
                            ins=[ag_in[t][:].opt()], outs=[ag_out[t][:].opt()],
                        )

            # ======== step_outT = fc_out_w.T @ [nodes3^T ; nodes0^T]  (+ bias)
            woutT = const.tile([P, KH, SH], F32, name="woutT")
            woutTb = const.tile([P, KH, SH], BF, name="woutTb")
            with tc.tile_pool(name="psF", bufs=1, space="PSUM") as psF:
                sp = [psF.tile([P, SH], F32, tag=f"sp{m}", name=f"sp{m}") for m in range(KH)]
                for k in range(KN):
                    fwt = wstream.tile([P, H], BF, tag="wt", name=f"fwt{k}")
                    nc.sync.dma_start(fwt[:], fcw_d[k * P:(k + 1) * P, :])
                    rhs_k = stateT[:, k, :] if k < KH else n0T_s[:, k - KH, :]
                    for m in range(KH):
                        nc.tensor.matmul(sp[m][:], lhsT=fwt[:, m * P:(m + 1) * P],
                                         rhs=rhs_k, start=(k == 0), stop=(k == KN - 1))
                for m in range(KH):
                    nc.vector.tensor_scalar_add(woutT[:, m, :], sp[m][:],
                                                fcb_s[:, m, None])
                    nc.vector.tensor_copy(woutTb[:, m, :], woutT[:, m, :])
            nc.sync.dma_start(wout_d.rearrange("(m p) s -> p m s", p=P), woutT[:])

            # ======== classifier: outT[SH, B] = weight_j.T @ x.T
            with tc.tile_pool(name="psC", bufs=2, space="PSUM") as psC, \
                 tc.tile_pool(name="xpool", bufs=2) as xpool:
                for nb in range(B // NB):
                    nbs = slice(nb * NB, (nb + 1) * NB)
                    xts = []
                    for k in range(KH):
                        xt = xpool.tile([P, NB], BF, tag=f"xt{k}", name=f"xt{nb}_{k}")
                        nc.sync.dma_start(xt[:], xT_d[k * P:(k + 1) * P, nbs])
                        xts.append(xt)
                    for mr in range(SM):
                        cp = psC.tile([P, NB], F32, tag=f"cp{mr}", name=f"cp{nb}_{mr}")
                        for k in range(KH):
                            nc.tensor.matmul(cp[:], lhsT=woutTb[:, k, mr * P:(mr + 1) * P],
                                             rhs=xts[k], start=(k == 0), stop=(k == KH - 1))
                        ot = work.tile([P, NB], F32, tag="ot", name=f"ot{nb}_{mr}")
                        nc.vector.tensor_copy(ot[:], cp[:])
                        nc.sync.dma_start(outT_d[mr * P:(mr + 1) * P, nbs], ot[:])

    nc.finalize()
    return nc


def _prep_inputs(x, last_fc_weight, in_matrix, w3w, w3u, w4w, w4u, w5w, w5u,
                 fc_out_w, fc_out_b):
    """Host-side sharding / layout glue. Returns in_maps for the 8 cores."""
    def c(a, dt=bf16):
        return np.ascontiguousarray(a).astype(dt)

    nodes0 = c(last_fc_weight.T)                    # [N, H] bf16
    xT = c(x.T)                                     # [H, B] bf16
    w3w_b, w4w_b, w5w_b = c(w3w), c(w4w), c(w5w)
    w3u_b, w4u_b, w5u_b = c(w3u), c(w4u), c(w5u)
    fcw_b = c(fc_out_w)
    fcb = np.ascontiguousarray(
        np.asarray(fc_out_b, dtype=np.float32).reshape(KH, P).T)  # [P, KH]

    in_maps = []
    for j in range(NCORES):
        rows = slice(j * SH, (j + 1) * SH)
        inb = np.concatenate(
            [np.ascontiguousarray(in_matrix[rows, :].T),
             np.ascontiguousarray(in_matrix[:, rows])], axis=1)   # [N, 2*SH]
        in_maps.append({
            "inb": c(inb),
            "nodes0": nodes0,
            "st0": np.ascontiguousarray(last_fc_weight[:, rows].T, dtype=np.float32),
            "n0T": c(last_fc_weight[:, rows]),
            "w3w": w3w_b, "w4w": w4w_b, "w5w": w5w_b,
            "w3u": w3u_b, "w4u": w4u_b, "w5u": w5u_b,
            "fcw": fcw_b, "fcb": fcb, "xT": xT,
        })
    return in_maps


def _run(in_maps, trace=False):
    if "nc" not in _CACHE:
        _CACHE["nc"] = _build()
    return bass_utils.run_bass_kernel_spmd(
        _CACHE["nc"], in_maps, core_ids=list(range(NCORES)), trace=trace)


def kernel(x, last_fc_weight, in_matrix, w3w, w3u, w4w, w4u, w5w, w5u,
           fc_out_w, fc_out_b):
    in_maps = _prep_inputs(x, last_fc_weight, in_matrix, w3w, w3u, w4w, w4u,
                           w5w, w5u, fc_out_w, fc_out_b)
    res = _run(in_maps)
    output = np.empty((B, N), dtype=np.float32)
    l2 = 0.0
    for j in range(NCORES):
        output[:, j * SH:(j + 1) * SH] = res.results[j]["outT"].T
        l2 += float(np.sum(res.results[j]["wout"].astype(np.float64) ** 2))
    return output, np.float32(l2)
